# revision 5
# baseline (speedup 1.0000x reference)
"""Trainium2 Bass kernel for nn_CrossAttnMLP (cross-attn + dual FFN + BN MLP head).

Sharding: pure data-parallel over 8 NeuronCores (batch 65536 -> 8 x 8192).

On-chip layout keeps features on the SBUF partition dim and batch on the free
dim, so every layer is matmul(lhsT=W^T, rhs=act) and layers chain with no
transposes; x is pre-transposed (and feature-padded to 896) on the host.
LayerNorm runs via PE projector matmuls: diff = (I - 11^T/128) @ z and
var = (11^T/128) @ diff^2, then r = rsqrt(var+eps) on ScalarE and a single
fused (diff*g)*r on VectorE. All affine biases are folded host-side into
per-partition bias vectors applied inside fused ACT/DVE ops.
BatchNorm uses exact full-batch stats: per-core sum/sumsq accumulate free via
activation accum_out, then one tiny AllReduce per BN layer (128x2 / 64x2).
Matmuls run in float32r (TF32, 1 cycle/row at N>=256) with fp32 PSUM.
"""
import sys, os
sys.path.insert(0, "/opt/trn_rl_repo")
import numpy as np
import concourse.bass as bass
import concourse.bacc as bacc
import concourse.tile as tile
from concourse import mybir
from concourse.bass_utils import run_bass_kernel_spmd

AF = mybir.ActivationFunctionType
ALU = mybir.AluOpType
F32 = mybir.dt.float32
F32R = mybir.dt.float32r

N_CORES = 8
B = 65536
PEP, TCR, D, FF = 384, 480, 128, 512
H1, H2 = 128, 64
EPS = 1e-5
XP = 896            # padded x feature dim (7 x 128)
NK = XP // 128      # 7 x-chunks
BC = B // N_CORES   # 8192 rows per core
N = 512             # batch columns per tile
NT = BC // N        # 16 tiles per core

# vecs ([128, 12] fp32) column indices
(C_BZ1P, C_BZ1T, C_G1P, C_G1T, C_BC2P, C_BC2T, C_G2P, C_G2T,
 C_BH1, C_BN1G, C_BN1B, C_PAD) = range(12)
# vech ([64, 4] fp32): 0=b_h2, 1=bn2_g, 2=bn2_b, 3=b_out(at row 0)

LAST_RESULT = None
_NC_CACHE = {}


def _build():
    nc = bacc.Bacc("TRN2", target_bir_lowering=False, debug=False,
                   enable_asserts=True, num_devices=N_CORES)

    def din(name, shape, dt=F32R):
        return nc.dram_tensor(name, shape, dt, kind="ExternalInput").ap()

    xt_d = din("xt", [XP, BC])
    wpep_d = din("wpepT", [PEP, D])
    wtcr_d = din("wtcrT", [512, D])            # padded 480 -> 512
    wap_d = din("wattnpT", [D, D])
    wat_d = din("wattntT", [D, D])
    q_d = din("qT", [D, D])
    pm_d = din("pT", [D, D])
    w1p_d = din("w1pT", [D, FF])
    w1t_d = din("w1tT", [D, FF])
    w2p_d = din("w2pT", [FF, D])
    w2t_d = din("w2tT", [FF, D])
    wh1p_d = din("wh1pT", [D, H1])
    wh1t_d = din("wh1tT", [D, H1])
    wh2_d = din("wh2T", [H1, H2])
    wout_d = din("woutT", [H2, 1])
    vecs_d = din("vecs", [D, 12], F32)
    vech_d = din("vech", [H2, 5], F32)
    bf1p_d = din("bf1p", [D, 4], F32)
    bf1t_d = din("bf1t", [D, 4], F32)
    y_d = nc.dram_tensor("y", [1, BC], F32, kind="ExternalOutput").ap()

    with tile.TileContext(nc) as tc:
        with tc.tile_pool(name="wpool", bufs=1) as wp, \
             tc.tile_pool(name="xpool", bufs=2) as xp, \
             tc.tile_pool(name="work", bufs=2) as wk, \
             tc.tile_pool(name="keep", bufs=1) as kp, \
             tc.tile_pool(name="ps1", bufs=1, space="PSUM") as ps1, \
             tc.tile_pool(name="ps2", bufs=2, space="PSUM") as ps2, \
             tc.tile_pool(name="dram", bufs=1, space="DRAM") as dr:

            # ---- load weights (once) ----
            def wtile(dram_ap, shape, tag, dt=F32R):
                t = wp.tile(shape, dt, tag=tag)
                nc.sync.dma_start(t[:], dram_ap)
                return t
            wpep = wtile(wpep_d.rearrange("(k p) m -> p k m", p=128), [128, 3, D], "wpep")
            wtcr = wtile(wtcr_d.rearrange("(k p) m -> p k m", p=128), [128, 4, D], "wtcr")
            wap = wtile(wap_d[:], [D, D], "wap")
            wat = wtile(wat_d[:], [D, D], "wat")
            qm = wtile(q_d[:], [D, D], "qm")
            pm = wtile(pm_d[:], [D, D], "pm")
            w1p = wtile(w1p_d[:], [D, FF], "w1p")
            w1t = wtile(w1t_d[:], [D, FF], "w1t")
            w2p = wtile(w2p_d.rearrange("(k p) m -> p k m", p=128), [128, 4, D], "w2p")
            w2t = wtile(w2t_d.rearrange("(k p) m -> p k m", p=128), [128, 4, D], "w2t")
            wh1p = wtile(wh1p_d[:], [D, H1], "wh1p")
            wh1t = wtile(wh1t_d[:], [D, H1], "wh1t")
            wh2 = wtile(wh2_d[:], [H1, H2], "wh2")
            wout = wtile(wout_d[:], [H2, 1], "wout")
            vecs = wtile(vecs_d[:], [D, 12], "vecs", F32)
            vech = wtile(vech_d[:], [H2, 5], "vech", F32)
            bf1p = wtile(bf1p_d[:], [D, 4], "bf1p", F32)
            bf1t = wtile(bf1t_d[:], [D, 4], "bf1t", F32)

            def vcol(c):
                return vecs[:, c:c + 1]

            # ---- retained activations + per-tile stats columns ----
            h1pre = kp.tile([D, NT, N], F32R, tag="h1pre")
            h2pre = kp.tile([H2, NT, N], F32R, tag="h2pre")
            s1c = kp.tile([D, NT], F32, tag="s1c")
            s2c = kp.tile([D, NT], F32, tag="s2c")
            u1c = kp.tile([H2, NT], F32, tag="u1c")
            u2c = kp.tile([H2, NT], F32, tag="u2c")

            xt_r = xt_d.rearrange("(k p) n -> p k n", p=128)

            def layernorm(z_sb, gcol, tag):
                """z_sb [128,N] f32r -> ln [128,N] f32r (= normalized * g)."""
                diff = ps2.tile([D, N], F32, tag="diff")
                nc.tensor.matmul(diff[:], qm[:], z_sb[:], start=True, stop=True)
                dsq = wk.tile([D, N], F32R, tag="dsq")
                nc.scalar.activation(dsq[:], diff[:], AF.Square)
                var = ps1.tile([D, N], F32, tag="var")
                nc.tensor.matmul(var[:], pm[:], dsq[:], start=True, stop=True)
                r = wk.tile([D, N], F32, tag="lnr")
                nc.scalar.activation(r[:], var[:], AF.Abs_reciprocal_sqrt, bias=vcol(C_PAD))
                ln = wk.tile([D, N], F32R, tag=tag)
                nc.vector.scalar_tensor_tensor(
                    ln[:], diff[:], gcol, r[:], op0=ALU.mult, op1=ALU.mult)
                return ln

            # =================== phase A ===================
            for i in range(NT):
                xt = xp.tile([128, NK, N], F32R, tag="xt")
                nc.sync.dma_start(xt[:], xt_r[:, :, i * N:(i + 1) * N])

                t_ps = ps1.tile([D, N], F32, tag="front_t")
                for k in range(4):
                    nc.tensor.matmul(t_ps[:], wtcr[:, k, :], xt[:, 3 + k, :],
                                     start=(k == 0), stop=False)
                tcr = wk.tile([D, N], F32R, tag="tcr")
                nc.scalar.copy(tcr[:], t_ps[:])

                p_ps = ps1.tile([D, N], F32, tag="front_p")
                for k in range(3):
                    nc.tensor.matmul(p_ps[:], wpep[:, k, :], xt[:, k, :],
                                     start=(k == 0), stop=False)
                pep = wk.tile([D, N], F32R, tag="pep")
                nc.scalar.copy(pep[:], p_ps[:])

                nc.tensor.matmul(p_ps[:], wap[:], tcr[:], start=False, stop=True)
                nc.tensor.matmul(t_ps[:], wat[:], pep[:], start=False, stop=True)

                z1p = wk.tile([D, N], F32R, tag="z1p")
                nc.vector.tensor_scalar_add(z1p[:], p_ps[:], vcol(C_BZ1P))
                z1t = wk.tile([D, N], F32R, tag="z1t")
                nc.vector.tensor_scalar_add(z1t[:], t_ps[:], vcol(C_BZ1T))

                ln1p = layernorm(z1p, vcol(C_G1P), "ln1p")
                ln1t = layernorm(z1t, vcol(C_G1T), "ln1t")

                def ffn(ln1, w1, w2, bf1):
                    z2ps = ps1.tile([D, N], F32, tag="z2acc")
                    for m in range(4):
                        h_ps = ps2.tile([D, N], F32, tag="h_ps")
                        nc.tensor.matmul(h_ps[:], w1[:, m * 128:(m + 1) * 128],
                                         ln1[:], start=True, stop=True)
                        hg = wk.tile([D, N], F32R, tag="hg")
                        nc.scalar.activation(hg[:], h_ps[:], AF.Gelu,
                                             bias=bf1[:, m:m + 1])
                        nc.tensor.matmul(z2ps[:], w2[:, m, :], hg[:],
                                         start=(m == 0), stop=(m == 3))
                    return z2ps

                z2p_ps = ffn(ln1p, w1p, w2p, bf1p)
                z2p = wk.tile([D, N], F32R, tag="z2p_sb")
                nc.vector.scalar_tensor_tensor(
                    z2p[:], z2p_ps[:], vcol(C_BC2P), ln1p[:].bitcast(F32),
                    op0=ALU.add, op1=ALU.add)
                ln2p = layernorm(z2p, vcol(C_G2P), "ln2p")

                z2t_ps = ffn(ln1t, w1t, w2t, bf1t)
                z2t = wk.tile([D, N], F32R, tag="z2t_sb")
                nc.vector.scalar_tensor_tensor(
                    z2t[:], z2t_ps[:], vcol(C_BC2T), ln1t[:].bitcast(F32),
                    op0=ALU.add, op1=ALU.add)
                ln2t = layernorm(z2t, vcol(C_G2T), "ln2t")

                h1_ps = ps1.tile([D, N], F32, tag="z2acc")
                nc.tensor.matmul(h1_ps[:], wh1p[:], ln2p[:], start=True, stop=False)
                nc.tensor.matmul(h1_ps[:], wh1t[:], ln2t[:], start=False, stop=True)
                nc.scalar.activation(h1pre[:, i, :], h1_ps[:], AF.Identity,
                                     bias=vcol(C_BH1), accum_out=s1c[:, i:i + 1])
                sq = wk.tile([D, N], F32, tag="sq")
                nc.scalar.activation(sq[:], h1pre[:, i, :].bitcast(F32), AF.Square,
                                     accum_out=s2c[:, i:i + 1])

            # ============ BN stats: reduce, allreduce, scale/shift ============
            def bn_stats(sc1, sc2, parts, g_ap, b_ap, eps_ap, tg):
                st = wk.tile([parts, 2], F32, tag=tg + "st")
                nc.vector.reduce_sum(st[:, 0:1], sc1[:], axis=mybir.AxisListType.X)
                nc.vector.reduce_sum(st[:, 1:2], sc2[:], axis=mybir.AxisListType.X)
                bin_t = dr.tile([parts, 2], F32, tag=tg + "i")
                bout_t = dr.tile([parts, 2], F32, tag=tg + "o")
                nc.sync.dma_start(bin_t[:], st[:])
                nc.gpsimd.collective_compute(
                    "AllReduce", ALU.add,
                    replica_groups=[list(range(N_CORES))],
                    ins=[bin_t.opt()], outs=[bout_t.opt()])
                g = wk.tile([parts, 2], F32, tag=tg + "g")
                nc.sync.dma_start(g[:], bout_t[:])
                mu = wk.tile([parts, 4], F32, tag=tg + "m")
                nc.vector.tensor_scalar_mul(mu[:, 0:2], g[:], 1.0 / B)  # mu | e
                nc.vector.tensor_tensor(mu[:, 2:3], mu[:, 0:1], mu[:, 0:1], ALU.mult)
                nc.vector.tensor_tensor(mu[:, 3:4], mu[:, 1:2], mu[:, 2:3],
                                        ALU.subtract)
                rb = wk.tile([parts, 3], F32, tag=tg + "r")
                nc.scalar.activation(rb[:, 0:1], mu[:, 3:4],
                                     AF.Abs_reciprocal_sqrt, bias=eps_ap)
                nc.vector.tensor_tensor(rb[:, 1:2], rb[:, 0:1], g_ap, ALU.mult)
                ms = wk.tile([parts, 1], F32, tag=tg + "x")
                nc.vector.tensor_tensor(ms[:], mu[:, 0:1], rb[:, 1:2], ALU.mult)
                nc.vector.tensor_tensor(rb[:, 2:3], b_ap, ms[:], ALU.subtract)
                return rb  # [:,1:2]=scale  [:,2:3]=shift

            bn1 = bn_stats(s1c, s2c, D, vcol(C_BN1G), vcol(C_BN1B), vcol(C_PAD), "bn1")

            # =================== phase C ===================
            for i in range(NT):
                h1g = wk.tile([D, N], F32R, tag="h1g")
                nc.scalar.activation(h1g[:], h1pre[:, i, :].bitcast(F32), AF.Gelu,
                                     scale=bn1[:, 1:2], bias=bn1[:, 2:3])
                h2_ps = ps1.tile([H2, N], F32, tag="var")
                nc.tensor.matmul(h2_ps[:], wh2[:], h1g[:], start=True, stop=True)
                nc.scalar.activation(h2pre[:, i, :], h2_ps[:], AF.Identity,
                                     bias=vech[:, 0:1], accum_out=u1c[:, i:i + 1])
                sq2 = wk.tile([H2, N], F32, tag="sq2")
                nc.scalar.activation(sq2[:], h2pre[:, i, :].bitcast(F32), AF.Square,
                                     accum_out=u2c[:, i:i + 1])

            bn2 = bn_stats(u1c, u2c, H2, vech[:, 1:2], vech[:, 2:3], vech[:, 4:5], "bn2")

            # =================== phase E ===================
            for i in range(NT):
                h2g = wk.tile([H2, N], F32R, tag="h2g")
                nc.scalar.activation(h2g[:], h2pre[:, i, :].bitcast(F32), AF.Gelu,
                                     scale=bn2[:, 1:2], bias=bn2[:, 2:3])
                o_ps = ps1.tile([1, N], F32, tag="var")
                nc.tensor.matmul(o_ps[:], wout[:], h2g[:], start=True, stop=True)
                osb = wk.tile([1, N], F32, tag="osb")
                nc.scalar.activation(osb[:], o_ps[:], AF.Identity,
                                     bias=vech[0:1, 3:4])
                nc.sync.dma_start(y_d[:, i * N:(i + 1) * N], osb[:])

    nc.compile()
    return nc


def _prep_inputs(inputs):
    """Host-side: fold biases, transpose/pad x, build per-core in_maps."""
    f64 = lambda a: np.asarray(a, dtype=np.float64)
    x = np.asarray(inputs["x"], dtype=np.float32)

    w_pep, b_pep = f64(inputs["w_pep"]), f64(inputs["b_pep"])
    w_tcr, b_tcr = f64(inputs["w_tcr"]), f64(inputs["b_tcr"])
    wv_p2t, bv_p2t = f64(inputs["wv_p2t"]), f64(inputs["bv_p2t"])
    wo_p2t, bo_p2t = f64(inputs["wo_p2t"]), f64(inputs["bo_p2t"])
    wv_t2p, bv_t2p = f64(inputs["wv_t2p"]), f64(inputs["bv_t2p"])
    wo_t2p, bo_t2p = f64(inputs["wo_t2p"]), f64(inputs["bo_t2p"])

    W_ap = wo_p2t @ wv_p2t                  # pa_raw = W_ap @ tcr + c_ap
    c_ap = wo_p2t @ bv_p2t + bo_p2t
    W_at = wo_t2p @ wv_t2p
    c_at = wo_t2p @ bv_t2p + bo_t2p

    bias_z1p = b_pep + W_ap @ b_tcr + c_ap
    bias_z1t = b_tcr + W_at @ b_pep + c_at

    ffn_w1p, ffn_b1p = f64(inputs["ffn_w1p"]), f64(inputs["ffn_b1p"])
    ffn_w2p, ffn_b2p = f64(inputs["ffn_w2p"]), f64(inputs["ffn_b2p"])
    ffn_w1t, ffn_b1t = f64(inputs["ffn_w1t"]), f64(inputs["ffn_b1t"])
    ffn_w2t, ffn_b2t = f64(inputs["ffn_w2t"]), f64(inputs["ffn_b2t"])
    ln_b1p, ln_b1t = f64(inputs["ln_b1p"]), f64(inputs["ln_b1t"])
    ln_b2p, ln_b2t = f64(inputs["ln_b2p"]), f64(inputs["ln_b2t"])

    bias_f1p = ffn_w1p @ ln_b1p + ffn_b1p   # [512]
    bias_f1t = ffn_w1t @ ln_b1t + ffn_b1t
    bias_c2p = ffn_b2p + ln_b1p             # residual + ffn2 bias
    bias_c2t = ffn_b2t + ln_b1t

    w_h1, b_h1 = f64(inputs["w_h1"]), f64(inputs["b_h1"])
    bias_h1 = w_h1[:, :D] @ ln_b2p + w_h1[:, D:] @ ln_b2t + b_h1

    f32c = lambda a: np.ascontiguousarray(a, dtype=np.float32)
    ones = np.full((D, D), 1.0 / D, dtype=np.float32)
    qmat = np.eye(D, dtype=np.float32) - ones

    vecs = np.zeros((D, 12), dtype=np.float32)
    vecs[:, C_BZ1P] = bias_z1p
    vecs[:, C_BZ1T] = bias_z1t
    vecs[:, C_G1P] = inputs["ln_g1p"]
    vecs[:, C_G1T] = inputs["ln_g1t"]
    vecs[:, C_BC2P] = bias_c2p
    vecs[:, C_BC2T] = bias_c2t
    vecs[:, C_G2P] = inputs["ln_g2p"]
    vecs[:, C_G2T] = inputs["ln_g2t"]
    vecs[:, C_BH1] = bias_h1
    vecs[:, C_BN1G] = inputs["bn1_g"]
    vecs[:, C_BN1B] = inputs["bn1_b"]
    vecs[:, C_PAD] = EPS

    vech = np.zeros((H2, 5), dtype=np.float32)
    vech[:, 4] = EPS
    vech[:, 0] = inputs["b_h2"]
    vech[:, 1] = inputs["bn2_g"]
    vech[:, 2] = inputs["bn2_b"]
    vech[0, 3] = float(np.asarray(inputs["b_out"]).reshape(-1)[0])

    wtcr_pad = np.zeros((512, D), dtype=np.float32)
    wtcr_pad[:TCR, :] = f32c(w_tcr.T)

    common = {
        "wpepT": f32c(w_pep.T),
        "wtcrT": wtcr_pad,
        "wattnpT": f32c(W_ap.T),
        "wattntT": f32c(W_at.T),
        "qT": qmat,
        "pT": ones,
        "w1pT": f32c(ffn_w1p.T),
        "w1tT": f32c(ffn_w1t.T),
        "w2pT": f32c(ffn_w2p.T),
        "w2tT": f32c(ffn_w2t.T),
        "wh1pT": f32c(w_h1[:, :D].T),
        "wh1tT": f32c(w_h1[:, D:].T),
        "wh2T": f32c(f64(inputs["w_h2"]).T),
        "woutT": f32c(f64(inputs["w_out"]).T),
        "vecs": vecs,
        "vech": vech,
        "bf1p": f32c(bias_f1p.reshape(4, 128).T),
        "bf1t": f32c(bias_f1t.reshape(4, 128).T),
    }
    in_maps = []
    for c in range(N_CORES):
        xs = x[c * BC:(c + 1) * BC]         # [8192, 864]
        xt = np.zeros((XP, BC), dtype=np.float32)
        xt[:PEP + TCR, :] = xs.T
        m = dict(common)
        m["xt"] = xt
        in_maps.append(m)
    return in_maps


def kernel(**inputs) -> np.ndarray:
    global LAST_RESULT
    if "nc" not in _NC_CACHE:
        _NC_CACHE["nc"] = _build()
    nc = _NC_CACHE["nc"]
    in_maps = _prep_inputs(inputs)
    res = run_bass_kernel_spmd(nc, in_maps, core_ids=list(range(N_CORES)))
    LAST_RESULT = res
    out = np.concatenate([res.results[c]["y"].reshape(BC) for c in range(N_CORES)])
    return out.reshape(B, 1).astype(np.float32)


if __name__ == "__main__":
    import time
    t0 = time.time()
    nc = _build()
    print(f"build + bacc compile OK in {time.time() - t0:.1f}s")
    from concourse.bass_utils import compile_bass_kernel
    import tempfile
    t0 = time.time()
    neff = compile_bass_kernel(nc, tempfile.mkdtemp())
    print(f"walrus compile OK in {time.time() - t0:.1f}s -> {neff}")
